# revision 27
# baseline (speedup 1.0000x reference)
"""AdaAT (per-channel affine warp + bilinear grid_sample) on 8 TRN2 NeuronCores.

Sharding: data-parallel over batch (B=8 -> 1 sample per core).
Per core: 256 channels of 128x128, each warped by its own
rotation/scale/translation and sampled bilinearly (border padding).

Bottleneck: GPSIMD ap_gather (~27.5 ns/idx/DSP-core, 1 idx per pixel,
8 channels in parallel per call) => ~14.4 ms floor per core. Everything
else is pipelined into the gather shadow:
  - S tiles hold 32 channels (4 shift-staggered copies x 8 groups x
    4 bands), loaded once per 4 gather calls on the Scalar queue.
  - Each call's gather is split into two 8192-idx halves so a single
    G buffer double-pumps (subtile deps let call k+1 half0 overwrite
    while half1 is still being consumed).
  - Index build on Vector, tap reshape DMAs on Sync, bilinear weights
    on Scalar+Vector, output stores on Tensor queue. GPSIMD runs only
    gathers.
"""

import numpy as np

B, D, H, W = 8, 256, 128, 128
NPIX = H * W  # 16384
NCORES = 8
PI = 3.14159  # matches reference
CALLS = D // 8  # 32 gather calls per core, 8 channels each
FM_PAD = 512
HALF = NPIX // 2  # 8192 idxs per gather half

_GRAPH_CACHE = {}


def _host_constants():
    """Input-independent constant tensors shipped to every core."""
    p = np.arange(128)
    s = np.arange(1024)
    # wrapped-16 iotas: pixel j = h*128+w lives at (partition j%16, free j//16)
    pix = s[None, :] * 16 + (p[:, None] % 16)  # [128, 1024]
    iw_wr = (pix % 128).astype(np.float32)
    ih_wr = (pix // 128).astype(np.float32)
    iw128 = np.broadcast_to(np.arange(128, dtype=np.float32), (128, 128)).copy()
    ident = np.eye(128, dtype=np.float32)
    # grouped-coefficient selection: grp[p, k] = coef[8k + p//16]
    c_all = np.arange(256)
    sel_lhsT = np.zeros((128, 256), dtype=np.float32)
    selmask = np.zeros((128, 64), dtype=np.float32)
    for chunk in range(2):
        c = chunk * 128 + np.arange(128)
        sel_lhsT[:, chunk * 128 : chunk * 128 + 128] = (
            (c[:, None] % 8) == (p[None, :] // 16)
        ).astype(np.float32)
        selmask[:, chunk * 32 : chunk * 32 + 32] = (
            (c[:, None] // 8) == np.arange(32)[None, :]
        ).astype(np.float32)
    lhsT2 = np.stack([p.astype(np.float32), np.ones(128, dtype=np.float32)])
    ones_row = np.ones((1, 128), dtype=np.float32)
    return dict(
        iw_wr=iw_wr, ih_wr=ih_wr, iw128=iw128, ident=ident,
        sel_lhsT=sel_lhsT, selmask=selmask, lhsT2=lhsT2, ones_row=ones_row,
    )


def _col2(x):
    """[256] -> [128, 2] column-chunk layout (chunk j in column j)."""
    return np.ascontiguousarray(x.reshape(2, 128).T)


def _mm_layout(Wm, n_out):
    """[256, n_out] -> [128, 2*n_out]: chunk k of the contraction dim at
    columns [k*n_out, (k+1)*n_out)."""
    return np.ascontiguousarray(
        Wm.reshape(2, 128, n_out).transpose(1, 0, 2).reshape(128, 2 * n_out)
    )


def _build(trace_label=""):
    import os
    import concourse.bass as bass
    import concourse.tile as tile
    from concourse import bacc, mybir
    from concourse.bass import ds

    f32, i32, i16 = mybir.dt.float32, mybir.dt.int32, mybir.dt.int16
    AF = mybir.ActivationFunctionType
    OP = mybir.AluOpType

    nc = bacc.Bacc("TRN2", target_bir_lowering=False, debug=False,
                   num_devices=NCORES)

    def din(name, shape):
        return nc.dram_tensor(name, list(shape), f32, kind="ExternalInput").ap()

    fm = din("fm", [D * NPIX + FM_PAD])
    pc = din("pc", [128, 2])
    w1 = din("w1", [128, 512])
    ws = din("ws", [128, 512])
    wr = din("wr", [128, 512])
    wt = din("wt", [128, 1024])
    b1 = din("b1", [128, 2])
    bs = din("bs", [128, 2])
    br = din("br", [128, 2])
    bt = din("bt", [128, 4])
    iw_wr_d = din("iw_wr", [128, 1024])
    ih_wr_d = din("ih_wr", [128, 1024])
    iw128_d = din("iw128", [128, 128])
    ident_d = din("ident", [128, 128])
    sel_lhsT_d = din("sel_lhsT", [128, 256])
    selmask_d = din("selmask", [128, 64])
    lhsT2_d = din("lhsT2", [2, 128])
    ones_row_d = din("ones_row", [1, 128])
    out_d = nc.dram_tensor("out", [D * NPIX], f32, kind="ExternalOutput").ap()

    with tile.TileContext(nc) as tc:
        with (
            tc.tile_pool(name="setup", bufs=1) as setup,
            tc.tile_pool(name="psum", bufs=1, space="PSUM") as psum,
            tc.tile_pool(name="src", bufs=1) as srcp,
            tc.tile_pool(name="gath", bufs=1) as gathp,
            tc.tile_pool(name="idx", bufs=1) as idxp,
            tc.tile_pool(name="bi16", bufs=2) as bi16p,
            tc.tile_pool(name="tap", bufs=1) as tapp,
            tc.tile_pool(name="wts", bufs=1) as wtsp,
        ):
            # ---- stage in small tensors ----
            def stage(ap_dram, shape, tag):
                t = setup.tile(list(shape), f32, tag=tag, name=tag)
                nc.gpsimd.dma_start(t[:], ap_dram[:])
                return t

            pc_t = stage(pc, [128, 2], "s_pc")
            w1_t = stage(w1, [128, 512], "s_w1")
            ws_t = stage(ws, [128, 512], "s_ws")
            wr_t = stage(wr, [128, 512], "s_wr")
            wt_t = stage(wt, [128, 1024], "s_wt")
            b1_t = stage(b1, [128, 2], "s_b1")
            bs_t = stage(bs, [128, 2], "s_bs")
            br_t = stage(br, [128, 2], "s_br")
            bt_t = stage(bt, [128, 4], "s_bt")
            iw_wr_t = stage(iw_wr_d, [128, 1024], "s_iw_wr")
            ih_wr_t = stage(ih_wr_d, [128, 1024], "s_ih_wr")
            iw128_t = stage(iw128_d, [128, 128], "s_iw128")
            ident_t = stage(ident_d, [128, 128], "s_ident")
            sel_lhsT_t = stage(sel_lhsT_d, [128, 256], "s_sel_lhsT")
            selmask_t = stage(selmask_d, [128, 64], "s_selmask")
            lhsT2_t = stage(lhsT2_d, [2, 128], "s_lhsT2")
            ones_row_t = stage(ones_row_d, [1, 128], "s_ones_row")

            # ---- param MLP in column layout ----
            p_sb = setup.tile([128, 2], f32, tag="p_sb", name="p_sb")

            def mlp_cols(w_tile, rhs_tile, bias_tile, n_chunks_out, func,
                         out_tile, scale=1.0, n_out_cols=256):
                for m in range(n_chunks_out):
                    ps = psum.tile([128, 1], f32, space="PSUM", tag="mlp_ps",
                                   name="mlp_ps", bufs=4)
                    for kk in range(2):
                        nc.tensor.matmul(
                            ps[:],
                            lhsT=w_tile[:, kk * n_out_cols + m * 128 :
                                        kk * n_out_cols + m * 128 + 128],
                            rhs=rhs_tile[:, kk : kk + 1],
                            start=(kk == 0), stop=(kk == 1),
                        )
                    nc.scalar.activation(out_tile[:, m : m + 1], ps[:], func,
                                         bias=bias_tile[:, m : m + 1],
                                         scale=scale)

            mlp_cols(w1_t, pc_t, b1_t, 2, AF.Relu, p_sb)
            sig_sb = setup.tile([128, 2], f32, tag="sig_sb", name="sig_sb")
            mlp_cols(ws_t, p_sb, bs_t, 2, AF.Sigmoid, sig_sb)
            tnh_sb = setup.tile([128, 2], f32, tag="tnh_sb", name="tnh_sb")
            mlp_cols(wr_t, p_sb, br_t, 2, AF.Tanh, tnh_sb)
            tt_sb = setup.tile([128, 4], f32, tag="tt_sb", name="tt_sb")
            mlp_cols(wt_t, p_sb, bt_t, 4, AF.Tanh, tt_sb, n_out_cols=512)

            half_pi = setup.tile([128, 1], f32, tag="half_pi", name="half_pi")
            nc.vector.memset(half_pi[:], PI / 2.0)
            zero_b = setup.tile([128, 1], f32, tag="zero_b", name="zero_b")
            nc.vector.memset(zero_b[:], 0.0)
            cs_sb = setup.tile([128, 2], f32, tag="cs_sb", name="cs_sb")
            sn_sb = setup.tile([128, 2], f32, tag="sn_sb", name="sn_sb")
            sh_sb = setup.tile([128, 2], f32, tag="sh_sb", name="sh_sb")
            for m in range(2):
                # sin LUT is only accurate on ~[-pi, pi]; cos via half-angle
                nc.scalar.activation(sn_sb[:, m : m + 1], tnh_sb[:, m : m + 1],
                                     AF.Sin, bias=zero_b[:], scale=PI)
                nc.scalar.activation(sh_sb[:, m : m + 1], tnh_sb[:, m : m + 1],
                                     AF.Sin, bias=zero_b[:], scale=PI / 2.0)
                nc.scalar.activation(sh_sb[:, m : m + 1], sh_sb[:, m : m + 1],
                                     AF.Square, bias=zero_b[:], scale=1.0)
                nc.vector.tensor_scalar(cs_sb[:, m : m + 1],
                                        sh_sb[:, m : m + 1], -2.0, 1.0,
                                        op0=OP.mult, op1=OP.add)

            # ---- affine coefficients (pixel space), packed per chunk:
            # P10[:, m*5 + {0:bx, 1:ex, 2:ax, 3:ey, 4:bxn}] ----
            P10 = setup.tile([128, 10], f32, tag="P10", name="P10")
            AXF = 256.0 / 127.0
            for m in range(2):
                o = m * 5
                csig = setup.tile([128, 1], f32, tag="csig", name="csig")
                ssig = setup.tile([128, 1], f32, tag="ssig", name="ssig")
                nc.vector.tensor_tensor(csig[:], cs_sb[:, m : m + 1],
                                        sig_sb[:, m : m + 1], op=OP.mult)
                nc.vector.tensor_tensor(ssig[:], sn_sb[:, m : m + 1],
                                        sig_sb[:, m : m + 1], op=OP.mult)
                nc.vector.tensor_scalar(P10[:, o + 2 : o + 3], csig[:], AXF,
                                        None, op0=OP.mult)
                nc.vector.tensor_scalar(P10[:, o : o + 1], ssig[:], -AXF,
                                        None, op0=OP.mult)
                nc.vector.tensor_scalar(P10[:, o + 4 : o + 5], ssig[:], AXF,
                                        None, op0=OP.mult)  # bxn = -bx
                e1 = setup.tile([128, 1], f32, tag="e1", name="e1")
                nc.vector.tensor_scalar(e1[:], tt_sb[:, m : m + 1], 64.0, 63.5,
                                        op0=OP.mult, op1=OP.add)
                e2 = setup.tile([128, 1], f32, tag="e2", name="e2")
                nc.vector.scalar_tensor_tensor(e2[:], csig[:], -128.0, e1[:],
                                               op0=OP.mult, op1=OP.add)
                nc.vector.scalar_tensor_tensor(P10[:, o + 1 : o + 2], ssig[:],
                                               128.0, e2[:],
                                               op0=OP.mult, op1=OP.add)
                f1 = setup.tile([128, 1], f32, tag="f1", name="f1")
                nc.vector.tensor_scalar(f1[:], tt_sb[:, m + 2 : m + 3], 64.0,
                                        63.5, op0=OP.mult, op1=OP.add)
                f2 = setup.tile([128, 1], f32, tag="f2", name="f2")
                nc.vector.scalar_tensor_tensor(f2[:], ssig[:], -128.0, f1[:],
                                               op0=OP.mult, op1=OP.add)
                nc.vector.scalar_tensor_tensor(P10[:, o + 3 : o + 4], csig[:],
                                               -128.0, f2[:],
                                               op0=OP.mult, op1=OP.add)

            # ---- grouped coefficient tables [128, 32] (for wrapped idx calc)
            grp = {}
            for name, t_idx in (("ax", 2), ("bx", 0), ("ex", 1), ("ey", 3),
                                ("bxn", 4)):
                g_ps = psum.tile([128, 32], f32, space="PSUM", tag="grp_ps",
                                 name="grp_ps")
                for m in range(2):
                    rhs = setup.tile([128, 32], f32, tag="grp_rhs",
                                     name="grp_rhs")
                    nc.vector.tensor_scalar(
                        rhs[:], selmask_t[:, m * 32 : m * 32 + 32],
                        P10[:, m * 5 + t_idx : m * 5 + t_idx + 1], None,
                        op0=OP.mult)
                    nc.tensor.matmul(g_ps[:],
                                     lhsT=sel_lhsT_t[:, m * 128 : m * 128 + 128],
                                     rhs=rhs[:], start=(m == 0), stop=(m == 1))
                g_sb = setup.tile([128, 32], f32, tag=f"grp_{name}",
                                  name=f"grp_{name}")
                nc.vector.tensor_copy(g_sb[:], g_ps[:])
                grp[name] = g_sb

            # ---- broadcast + additive tables [128, 256] ----
            ax_b = setup.tile([128, 256], f32, tag="ax_b", name="ax_b")
            bxn_b = setup.tile([128, 256], f32, tag="bxn_b", name="bxn_b")
            bxh_ex = setup.tile([128, 256], f32, tag="bxh_ex", name="bxh_ex")
            axh_ey = setup.tile([128, 256], f32, tag="axh_ey", name="axh_ey")
            for m in range(2):
                o = m * 5
                sl = ds(m * 128, 128)

                def row_of(cols, ncol, tag):
                    tp = psum.tile([ncol, 128], f32, space="PSUM", tag="tp",
                                   name="tp")
                    nc.tensor.transpose(tp[:], P10[:, cols], identity=ident_t[:])
                    rs = setup.tile([ncol, 128], f32, tag=f"row_{tag}",
                                    name=f"row_{tag}")
                    nc.vector.tensor_copy(rs[:], tp[:])
                    return rs

                r_bx_ex = row_of(ds(o, 2), 2, "bxex")
                r_ax_ey = row_of(ds(o + 2, 2), 2, "axey")
                r_ax = row_of(ds(o + 2, 1), 1, "ax")
                r_bxn = row_of(ds(o + 4, 1), 1, "bxn")

                for dst, lhsT_ap, rhs_t in (
                    (bxh_ex, lhsT2_t[:], r_bx_ex),
                    (axh_ey, lhsT2_t[:], r_ax_ey),
                    (ax_b, ones_row_t[:], r_ax),
                    (bxn_b, ones_row_t[:], r_bxn),
                ):
                    pp = psum.tile([128, 128], f32, space="PSUM", tag="tab_ps",
                                   name="tab_ps")
                    nc.tensor.matmul(pp[:], lhsT=lhsT_ap, rhs=rhs_t[:],
                                     start=True, stop=True)
                    nc.vector.tensor_copy(dst[:, sl], pp[:])

            # ================= main loop =================
            REPEAT = int(os.environ.get("KREPEAT", "1"))

            def load_s_tile(kp):
                """S tile for calls 4kp..4kp+3: band j holds channels
                8(4kp+j)+g at partitions 16g+4j+t (t = shift index)."""
                S = srcp.tile([128, NPIX], f32, tag="S", name="S")
                for j in range(4):
                    for t_i, sh in enumerate((0, 1, W, W + 1)):
                        nc.gpsimd.dma_start(
                            S[:][4 * j + t_i :: 16, :],
                            fm[ds((32 * kp + 8 * j) * NPIX + sh, 8 * NPIX)]
                            .rearrange("(g i) -> g i", g=8),
                        )
                return S

            def build_idx(q):
                """bi16 [128, 1024] i16: wrapped-16 flat base indices for the
                8 channels of call q."""
                kk = ds(q, 1)
                w0 = idxp.tile([128, 1024], f32, tag="w0", name="w0")
                w1t = idxp.tile([128, 1024], f32, tag="w1", name="w1t")
                w2 = idxp.tile([128, 1024], i32, tag="w2", name="w2")
                w3 = idxp.tile([128, 1024], f32, tag="w3", name="w3")

                def axis(iw, ih, ga, gb, ge, qf_t):
                    # clipped affine -> exact floor (rounding-proof via is_gt)
                    nc.vector.tensor_scalar(w0[:], iw[:], grp[ga][:, kk], None,
                                            op0=OP.mult)
                    nc.vector.scalar_tensor_tensor(w0[:], ih[:],
                                                   grp[gb][:, kk], w0[:],
                                                   op0=OP.mult, op1=OP.add)
                    nc.vector.tensor_scalar(w0[:], w0[:], grp[ge][:, kk], 0.0,
                                            op0=OP.add, op1=OP.max)
                    nc.vector.tensor_scalar(w0[:], w0[:], 127.0, None,
                                            op0=OP.min)
                    nc.vector.tensor_copy(w2[:], w0[:])      # int(trunc)
                    nc.vector.tensor_copy(qf_t[:], w2[:])    # back to f32
                    w2f = w2.bitcast(f32)
                    nc.vector.tensor_tensor(w2f[:], qf_t[:], w0[:],
                                            op=OP.is_gt)
                    nc.vector.tensor_tensor(qf_t[:], qf_t[:], w2f[:],
                                            op=OP.subtract)
                    return qf_t

                x0f = axis(iw_wr_t, ih_wr_t, "ax", "bx", "ex", w1t)
                y0f = axis(iw_wr_t, ih_wr_t, "bxn", "ax", "ey", w3)
                # base = y0*128 + x0 (exact integers in f32)
                nc.vector.scalar_tensor_tensor(w0[:], y0f[:], 128.0, x0f[:],
                                               op0=OP.mult, op1=OP.add)
                nc.vector.tensor_copy(w2[:], w0[:])
                bi16 = bi16p.tile([128, 1024], i16, tag="bi16", name="bi16")
                nc.vector.tensor_copy(bi16[:], w2[:])
                return bi16

            def postprocess(q, G):
                """Reshape taps, compute bilinear weights, lerp, store: the
                8 channels of call q.

                All half-0 tap reshapes are emitted before any half-1 ones,
                so the Sync DMA queue drains the half-0 reads of G during
                gather(q, h1) and the next call's half-0 gather can start
                immediately after."""
                j = q % 4
                T4s = [tapp.tile([128, 512], f32, tag=f"T4_{g}", name="T4")
                       for g in range(8)]
                for h in range(2):
                    for g in range(8):
                        p0 = 16 * g + 4 * j
                        for t_i in range(4):
                            nc.sync.dma_start(
                                T4s[g][64 * h : 64 * h + 64,
                                       ds(t_i * 128, 128)],
                                G[h][p0 + t_i : p0 + t_i + 1, :],
                            )
                for g in range(8):
                    c = 8 * q + g
                    cc = ds(c, 1)
                    T4 = T4s[g]
                    T = [T4[:, ds(t_i * 128, 128)] for t_i in range(4)]

                    def coords_q(scale_t, bias_t, u0, u1, u2):
                        nc.scalar.activation(u0[:], iw128_t[:], AF.Identity,
                                             bias=bias_t[:, cc],
                                             scale=scale_t[:, cc])
                        nc.vector.tensor_scalar(u0[:], u0[:], 0.0, 127.0,
                                                op0=OP.max, op1=OP.min)
                        nc.vector.tensor_copy(u2[:], u0[:])   # i32 trunc
                        nc.vector.tensor_copy(u1[:], u2[:])   # f32
                        u2f = u2.bitcast(f32)
                        nc.vector.tensor_tensor(u2f[:], u1[:], u0[:],
                                                op=OP.is_gt)
                        nc.vector.tensor_tensor(u1[:], u1[:], u2f[:],
                                                op=OP.subtract)  # floor
                        nc.vector.tensor_tensor(u0[:], u0[:], u1[:],
                                                op=OP.subtract)  # frac
                        return u0

                    u0 = wtsp.tile([128, 128], f32, tag="u0", name="u0")
                    u1 = wtsp.tile([128, 128], f32, tag="u1", name="u1")
                    u2 = wtsp.tile([128, 128], i32, tag="u2", name="u2")
                    fx = coords_q(ax_b, bxh_ex, u0, u1, u2)
                    u3 = wtsp.tile([128, 128], f32, tag="u3", name="u3")
                    u4 = wtsp.tile([128, 128], f32, tag="u4", name="u4")
                    u5 = wtsp.tile([128, 128], i32, tag="u5", name="u5")
                    fy = coords_q(bxn_b, axh_ey, u3, u4, u5)

                    gx0 = wtsp.tile([128, 128], f32, tag="gx0", name="gx0")
                    nc.vector.tensor_scalar(gx0[:], fx[:], -1.0, 1.0,
                                            op0=OP.mult, op1=OP.add)
                    gy0 = wtsp.tile([128, 128], f32, tag="gy0", name="gy0")
                    nc.vector.tensor_scalar(gy0[:], fy[:], -1.0, 1.0,
                                            op0=OP.mult, op1=OP.add)

                    r0 = wtsp.tile([128, 128], f32, tag="r0", name="r0")
                    r1 = wtsp.tile([128, 128], f32, tag="r1", name="r1")
                    r2 = wtsp.tile([128, 128], f32, tag="r2", name="r2")
                    # top = T0*(1-fx) + T1*fx
                    nc.vector.tensor_tensor(r0[:], T[0], gx0[:], op=OP.mult)
                    nc.vector.tensor_tensor(r1[:], T[1], fx[:], op=OP.mult)
                    nc.vector.tensor_tensor(r0[:], r0[:], r1[:], op=OP.add)
                    # bot = T2*(1-fx) + T3*fx
                    nc.vector.tensor_tensor(r2[:], T[2], gx0[:], op=OP.mult)
                    nc.vector.tensor_tensor(r1[:], T[3], fx[:], op=OP.mult)
                    nc.vector.tensor_tensor(r2[:], r2[:], r1[:], op=OP.add)
                    # out = top*(1-fy) + bot*fy
                    nc.vector.tensor_tensor(r0[:], r0[:], gy0[:], op=OP.mult)
                    nc.vector.tensor_tensor(r2[:], r2[:], fy[:], op=OP.mult)
                    nc.vector.tensor_tensor(r0[:], r0[:], r2[:], op=OP.add)
                    nc.scalar.dma_start(
                        out_d[ds(c * NPIX, NPIX)].rearrange("(a b) -> a b",
                                                            a=128),
                        r0[:],
                    )

            SKIP_POST = os.environ.get("KSKIPPOST") == "1"  # timing probe
            for _rep in range(REPEAT):
                # Two half-G buffers: call q+1's half-0 gather overwrites G0
                # as soon as call q's half-0 tap reshapes have drained, while
                # half-1 is still being consumed from G1.
                G = [gathp.tile([128, HALF], f32, tag="G0", name="G0"),
                     gathp.tile([128, HALF], f32, tag="G1", name="G1")]
                S = None
                bi16 = None
                S_box = [load_s_tile(0)]
                for q in range(CALLS):
                    S = S_box[0]
                    if q == 0:
                        bi16 = build_idx(0)
                    for h in range(2):
                        nc.gpsimd.ap_gather(
                            G[h][:, :], S[:],
                            bi16[:, h * 512 : (h + 1) * 512],
                            channels=128, num_elems=NPIX, d=1, num_idxs=HALF)
                    # Next S tile: dispatched on the GPSIMD queue directly
                    # after this tile's last gather — queue order gives the
                    # WAR for free and the transfers spread across GPSIMD's
                    # full DMA ring set.
                    if (q + 1) % 4 == 0 and q + 1 < CALLS:
                        S_box[0] = load_s_tile((q + 1) // 4)
                    if q + 1 < CALLS:
                        bi16 = build_idx(q + 1)
                    if not SKIP_POST:
                        postprocess(q, G)

    nc.compile()
    return nc


def _prepare_in_maps(feature_map, para_code, W1, b1, Ws, bs, Wr, br, Wt, bt):
    consts = _host_constants()
    Wt_re = np.concatenate([Wt[:, 0::2], Wt[:, 1::2]], axis=1)
    bt_re = np.concatenate([bt[0::2], bt[1::2]])
    common = dict(
        w1=_mm_layout(W1, 256), ws=_mm_layout(Ws, 256), wr=_mm_layout(Wr, 256),
        wt=_mm_layout(Wt_re, 512),
        b1=_col2(b1), bs=_col2(bs), br=_col2(br),
        bt=np.ascontiguousarray(bt_re.reshape(4, 128).T),
        **consts,
    )
    common = {k: np.ascontiguousarray(v, dtype=np.float32)
              for k, v in common.items()}
    in_maps = []
    for i in range(NCORES):
        fm_i = np.concatenate([
            feature_map[i].reshape(-1),
            np.zeros(FM_PAD, dtype=np.float32),
        ])
        m = dict(common)
        m["fm"] = fm_i
        m["pc"] = _col2(para_code[i])
        in_maps.append(m)
    return in_maps


def _run(inputs, trace=False):
    from concourse.bass_utils import run_bass_kernel_spmd

    if "nc" not in _GRAPH_CACHE:
        _GRAPH_CACHE["nc"] = _build()
    nc = _GRAPH_CACHE["nc"]
    in_maps = _prepare_in_maps(**inputs)
    res = run_bass_kernel_spmd(nc, in_maps, core_ids=list(range(NCORES)),
                               trace=trace)
    out = np.stack([
        np.asarray(res.results[i]["out"]).reshape(D, H, W)
        for i in range(NCORES)
    ])
    return out, res


def kernel(**inputs) -> np.ndarray:
    out, _ = _run(inputs, trace=False)
    return out


# revision 33
# speedup vs baseline: 1.0677x; 1.0677x over previous
"""AdaAT (per-channel affine warp + bilinear grid_sample) on 8 TRN2 NeuronCores.

Sharding: data-parallel over batch (B=8 -> 1 sample per core).
Per core: 256 channels of 128x128, each warped by its own
rotation/scale/translation and sampled bilinearly (border padding).

Bottleneck: GPSIMD ap_gather (~27.5 ns/idx/DSP-core, 1 idx per pixel,
8 channels in parallel per call) => ~14.4 ms floor per core. Everything
else is pipelined into the gather shadow:
  - S tiles hold 32 channels (4 shift-staggered copies x 8 groups x
    4 bands), loaded once per 4 gather calls on the Scalar queue.
  - Each call's gather is split into two 8192-idx halves so a single
    G buffer double-pumps (subtile deps let call k+1 half0 overwrite
    while half1 is still being consumed).
  - Index build on Vector, tap reshape DMAs on Sync, bilinear weights
    on Scalar+Vector, output stores on Tensor queue. GPSIMD runs only
    gathers.
"""

import numpy as np

B, D, H, W = 8, 256, 128, 128
NPIX = H * W  # 16384
NCORES = 8
PI = 3.14159  # matches reference
CALLS = D // 8  # 32 gather calls per core, 8 channels each
FM_PAD = 512
HALF = NPIX // 2  # 8192 idxs per gather half

_GRAPH_CACHE = {}


def _host_constants():
    """Input-independent constant tensors shipped to every core."""
    p = np.arange(128)
    s = np.arange(1024)
    # wrapped-16 iotas: pixel j = h*128+w lives at (partition j%16, free j//16)
    pix = s[None, :] * 16 + (p[:, None] % 16)  # [128, 1024]
    iw_wr = (pix % 128).astype(np.float32)
    ih_wr = (pix // 128).astype(np.float32)
    iw128 = np.broadcast_to(np.arange(128, dtype=np.float32), (128, 128)).copy()
    ident = np.eye(128, dtype=np.float32)
    # grouped-coefficient selection: grp[p, k] = coef[8k + p//16]
    c_all = np.arange(256)
    sel_lhsT = np.zeros((128, 256), dtype=np.float32)
    selmask = np.zeros((128, 64), dtype=np.float32)
    for chunk in range(2):
        c = chunk * 128 + np.arange(128)
        sel_lhsT[:, chunk * 128 : chunk * 128 + 128] = (
            (c[:, None] % 8) == (p[None, :] // 16)
        ).astype(np.float32)
        selmask[:, chunk * 32 : chunk * 32 + 32] = (
            (c[:, None] // 8) == np.arange(32)[None, :]
        ).astype(np.float32)
    lhsT2 = np.stack([p.astype(np.float32), np.ones(128, dtype=np.float32)])
    ones_row = np.ones((1, 128), dtype=np.float32)
    return dict(
        iw_wr=iw_wr, ih_wr=ih_wr, iw128=iw128, ident=ident,
        sel_lhsT=sel_lhsT, selmask=selmask, lhsT2=lhsT2, ones_row=ones_row,
    )


def _col2(x):
    """[256] -> [128, 2] column-chunk layout (chunk j in column j)."""
    return np.ascontiguousarray(x.reshape(2, 128).T)


def _mm_layout(Wm, n_out):
    """[256, n_out] -> [128, 2*n_out]: chunk k of the contraction dim at
    columns [k*n_out, (k+1)*n_out)."""
    return np.ascontiguousarray(
        Wm.reshape(2, 128, n_out).transpose(1, 0, 2).reshape(128, 2 * n_out)
    )


def _build(trace_label=""):
    import os
    import concourse.bass as bass
    import concourse.tile as tile
    from concourse import bacc, mybir
    from concourse.bass import ds

    f32, i32, i16 = mybir.dt.float32, mybir.dt.int32, mybir.dt.int16
    AF = mybir.ActivationFunctionType
    OP = mybir.AluOpType

    nc = bacc.Bacc("TRN2", target_bir_lowering=False, debug=False,
                   num_devices=NCORES)

    bf16 = mybir.dt.bfloat16

    def din(name, shape, dtype=f32):
        return nc.dram_tensor(name, list(shape), dtype,
                              kind="ExternalInput").ap()

    # Host-packed overlapped bf16 pairs: unit j = (flat[j], flat[j+1]).
    # One d=2 gather at base b fetches both x-taps of a row; the shift-128
    # partition copy supplies the second row. Pairs are bit-exact DMA moves;
    # only the tap values themselves are rounded to bf16.
    fm = din("fm", [2 * (D * NPIX + 256)], dtype=bf16)
    pc = din("pc", [128, 2])
    w1 = din("w1", [128, 512])
    ws = din("ws", [128, 512])
    wr = din("wr", [128, 512])
    wt = din("wt", [128, 1024])
    b1 = din("b1", [128, 2])
    bs = din("bs", [128, 2])
    br = din("br", [128, 2])
    bt = din("bt", [128, 4])
    iw_wr_d = din("iw_wr", [128, 1024])
    ih_wr_d = din("ih_wr", [128, 1024])
    iw128_d = din("iw128", [128, 128])
    ident_d = din("ident", [128, 128])
    sel_lhsT_d = din("sel_lhsT", [128, 256])
    selmask_d = din("selmask", [128, 64])
    lhsT2_d = din("lhsT2", [2, 128])
    ones_row_d = din("ones_row", [1, 128])
    out_d = nc.dram_tensor("out", [D * NPIX], f32, kind="ExternalOutput").ap()

    with tile.TileContext(nc) as tc:
        with (
            tc.tile_pool(name="setup", bufs=1) as setup,
            tc.tile_pool(name="psum", bufs=1, space="PSUM") as psum,
            tc.tile_pool(name="src", bufs=1) as srcp,
            tc.tile_pool(name="gath", bufs=1) as gathp,
            tc.tile_pool(name="idx", bufs=1) as idxp,
            tc.tile_pool(name="bi16", bufs=2) as bi16p,
            tc.tile_pool(name="tap", bufs=1) as tapp,
            tc.tile_pool(name="wts", bufs=1) as wtsp,
        ):
            # ---- stage in small tensors ----
            def stage(ap_dram, shape, tag):
                t = setup.tile(list(shape), f32, tag=tag, name=tag)
                nc.gpsimd.dma_start(t[:], ap_dram[:])
                return t

            pc_t = stage(pc, [128, 2], "s_pc")
            w1_t = stage(w1, [128, 512], "s_w1")
            ws_t = stage(ws, [128, 512], "s_ws")
            wr_t = stage(wr, [128, 512], "s_wr")
            wt_t = stage(wt, [128, 1024], "s_wt")
            b1_t = stage(b1, [128, 2], "s_b1")
            bs_t = stage(bs, [128, 2], "s_bs")
            br_t = stage(br, [128, 2], "s_br")
            bt_t = stage(bt, [128, 4], "s_bt")
            iw_wr_t = stage(iw_wr_d, [128, 1024], "s_iw_wr")
            ih_wr_t = stage(ih_wr_d, [128, 1024], "s_ih_wr")
            iw128_t = stage(iw128_d, [128, 128], "s_iw128")
            ident_t = stage(ident_d, [128, 128], "s_ident")
            sel_lhsT_t = stage(sel_lhsT_d, [128, 256], "s_sel_lhsT")
            selmask_t = stage(selmask_d, [128, 64], "s_selmask")
            lhsT2_t = stage(lhsT2_d, [2, 128], "s_lhsT2")
            ones_row_t = stage(ones_row_d, [1, 128], "s_ones_row")

            # ---- param MLP in column layout ----
            p_sb = setup.tile([128, 2], f32, tag="p_sb", name="p_sb")

            def mlp_cols(w_tile, rhs_tile, bias_tile, n_chunks_out, func,
                         out_tile, scale=1.0, n_out_cols=256):
                for m in range(n_chunks_out):
                    ps = psum.tile([128, 1], f32, space="PSUM", tag="mlp_ps",
                                   name="mlp_ps", bufs=4)
                    for kk in range(2):
                        nc.tensor.matmul(
                            ps[:],
                            lhsT=w_tile[:, kk * n_out_cols + m * 128 :
                                        kk * n_out_cols + m * 128 + 128],
                            rhs=rhs_tile[:, kk : kk + 1],
                            start=(kk == 0), stop=(kk == 1),
                        )
                    nc.scalar.activation(out_tile[:, m : m + 1], ps[:], func,
                                         bias=bias_tile[:, m : m + 1],
                                         scale=scale)

            mlp_cols(w1_t, pc_t, b1_t, 2, AF.Relu, p_sb)
            sig_sb = setup.tile([128, 2], f32, tag="sig_sb", name="sig_sb")
            mlp_cols(ws_t, p_sb, bs_t, 2, AF.Sigmoid, sig_sb)
            tnh_sb = setup.tile([128, 2], f32, tag="tnh_sb", name="tnh_sb")
            mlp_cols(wr_t, p_sb, br_t, 2, AF.Tanh, tnh_sb)
            tt_sb = setup.tile([128, 4], f32, tag="tt_sb", name="tt_sb")
            mlp_cols(wt_t, p_sb, bt_t, 4, AF.Tanh, tt_sb, n_out_cols=512)

            half_pi = setup.tile([128, 1], f32, tag="half_pi", name="half_pi")
            nc.vector.memset(half_pi[:], PI / 2.0)
            zero_b = setup.tile([128, 1], f32, tag="zero_b", name="zero_b")
            nc.vector.memset(zero_b[:], 0.0)
            cs_sb = setup.tile([128, 2], f32, tag="cs_sb", name="cs_sb")
            sn_sb = setup.tile([128, 2], f32, tag="sn_sb", name="sn_sb")
            sh_sb = setup.tile([128, 2], f32, tag="sh_sb", name="sh_sb")
            for m in range(2):
                # sin LUT is only accurate on ~[-pi, pi]; cos via half-angle
                nc.scalar.activation(sn_sb[:, m : m + 1], tnh_sb[:, m : m + 1],
                                     AF.Sin, bias=zero_b[:], scale=PI)
                nc.scalar.activation(sh_sb[:, m : m + 1], tnh_sb[:, m : m + 1],
                                     AF.Sin, bias=zero_b[:], scale=PI / 2.0)
                nc.scalar.activation(sh_sb[:, m : m + 1], sh_sb[:, m : m + 1],
                                     AF.Square, bias=zero_b[:], scale=1.0)
                nc.vector.tensor_scalar(cs_sb[:, m : m + 1],
                                        sh_sb[:, m : m + 1], -2.0, 1.0,
                                        op0=OP.mult, op1=OP.add)

            # ---- affine coefficients (pixel space), packed per chunk:
            # P10[:, m*5 + {0:bx, 1:ex, 2:ax, 3:ey, 4:bxn}] ----
            P10 = setup.tile([128, 10], f32, tag="P10", name="P10")
            AXF = 256.0 / 127.0
            for m in range(2):
                o = m * 5
                csig = setup.tile([128, 1], f32, tag="csig", name="csig")
                ssig = setup.tile([128, 1], f32, tag="ssig", name="ssig")
                nc.vector.tensor_tensor(csig[:], cs_sb[:, m : m + 1],
                                        sig_sb[:, m : m + 1], op=OP.mult)
                nc.vector.tensor_tensor(ssig[:], sn_sb[:, m : m + 1],
                                        sig_sb[:, m : m + 1], op=OP.mult)
                nc.vector.tensor_scalar(P10[:, o + 2 : o + 3], csig[:], AXF,
                                        None, op0=OP.mult)
                nc.vector.tensor_scalar(P10[:, o : o + 1], ssig[:], -AXF,
                                        None, op0=OP.mult)
                nc.vector.tensor_scalar(P10[:, o + 4 : o + 5], ssig[:], AXF,
                                        None, op0=OP.mult)  # bxn = -bx
                e1 = setup.tile([128, 1], f32, tag="e1", name="e1")
                nc.vector.tensor_scalar(e1[:], tt_sb[:, m : m + 1], 64.0, 63.5,
                                        op0=OP.mult, op1=OP.add)
                e2 = setup.tile([128, 1], f32, tag="e2", name="e2")
                nc.vector.scalar_tensor_tensor(e2[:], csig[:], -128.0, e1[:],
                                               op0=OP.mult, op1=OP.add)
                nc.vector.scalar_tensor_tensor(P10[:, o + 1 : o + 2], ssig[:],
                                               128.0, e2[:],
                                               op0=OP.mult, op1=OP.add)
                f1 = setup.tile([128, 1], f32, tag="f1", name="f1")
                nc.vector.tensor_scalar(f1[:], tt_sb[:, m + 2 : m + 3], 64.0,
                                        63.5, op0=OP.mult, op1=OP.add)
                f2 = setup.tile([128, 1], f32, tag="f2", name="f2")
                nc.vector.scalar_tensor_tensor(f2[:], ssig[:], -128.0, f1[:],
                                               op0=OP.mult, op1=OP.add)
                nc.vector.scalar_tensor_tensor(P10[:, o + 3 : o + 4], csig[:],
                                               -128.0, f2[:],
                                               op0=OP.mult, op1=OP.add)

            # ---- grouped coefficient tables [128, 32] (for wrapped idx calc)
            grp = {}
            for name, t_idx in (("ax", 2), ("bx", 0), ("ex", 1), ("ey", 3),
                                ("bxn", 4)):
                g_ps = psum.tile([128, 32], f32, space="PSUM", tag="grp_ps",
                                 name="grp_ps")
                for m in range(2):
                    rhs = setup.tile([128, 32], f32, tag="grp_rhs",
                                     name="grp_rhs")
                    nc.vector.tensor_scalar(
                        rhs[:], selmask_t[:, m * 32 : m * 32 + 32],
                        P10[:, m * 5 + t_idx : m * 5 + t_idx + 1], None,
                        op0=OP.mult)
                    nc.tensor.matmul(g_ps[:],
                                     lhsT=sel_lhsT_t[:, m * 128 : m * 128 + 128],
                                     rhs=rhs[:], start=(m == 0), stop=(m == 1))
                g_sb = setup.tile([128, 32], f32, tag=f"grp_{name}",
                                  name=f"grp_{name}")
                nc.vector.tensor_copy(g_sb[:], g_ps[:])
                grp[name] = g_sb

            # ---- broadcast + additive tables [128, 256] ----
            ax_b = setup.tile([128, 256], f32, tag="ax_b", name="ax_b")
            bxn_b = setup.tile([128, 256], f32, tag="bxn_b", name="bxn_b")
            bxh_ex = setup.tile([128, 256], f32, tag="bxh_ex", name="bxh_ex")
            axh_ey = setup.tile([128, 256], f32, tag="axh_ey", name="axh_ey")
            for m in range(2):
                o = m * 5
                sl = ds(m * 128, 128)

                def row_of(cols, ncol, tag):
                    tp = psum.tile([ncol, 128], f32, space="PSUM", tag="tp",
                                   name="tp")
                    nc.tensor.transpose(tp[:], P10[:, cols], identity=ident_t[:])
                    rs = setup.tile([ncol, 128], f32, tag=f"row_{tag}",
                                    name=f"row_{tag}")
                    nc.vector.tensor_copy(rs[:], tp[:])
                    return rs

                r_bx_ex = row_of(ds(o, 2), 2, "bxex")
                r_ax_ey = row_of(ds(o + 2, 2), 2, "axey")
                r_ax = row_of(ds(o + 2, 1), 1, "ax")
                r_bxn = row_of(ds(o + 4, 1), 1, "bxn")

                for dst, lhsT_ap, rhs_t in (
                    (bxh_ex, lhsT2_t[:], r_bx_ex),
                    (axh_ey, lhsT2_t[:], r_ax_ey),
                    (ax_b, ones_row_t[:], r_ax),
                    (bxn_b, ones_row_t[:], r_bxn),
                ):
                    pp = psum.tile([128, 128], f32, space="PSUM", tag="tab_ps",
                                   name="tab_ps")
                    nc.tensor.matmul(pp[:], lhsT=lhsT_ap, rhs=rhs_t[:],
                                     start=True, stop=True)
                    nc.vector.tensor_copy(dst[:, sl], pp[:])

            # ================= main loop =================
            REPEAT = int(os.environ.get("KREPEAT", "1"))

            def load_s_tile(kp):
                """S tile for calls 8kp..8kp+7: band s holds channels
                8(8kp+s)+g as bf16 pair-units at partitions 16g+2s+b
                (b=0: shift 0 -> (v00,v01); b=1: shift W -> (v10,v11))."""
                S = srcp.tile([128, 2 * NPIX], bf16, tag="S", name="S")
                for s in range(8):
                    for b in range(2):
                        nc.gpsimd.dma_start(
                            S[:][2 * s + b :: 16, :],
                            fm[ds(2 * ((64 * kp + 8 * s) * NPIX + 128 * b),
                                  2 * 8 * NPIX)]
                            .rearrange("(g i) -> g i", g=8),
                        )
                return S

            def build_idx(q):
                """bi16 [128, 1024] i16: wrapped-16 flat base indices for the
                8 channels of call q."""
                kk = ds(q, 1)
                w0 = idxp.tile([128, 1024], f32, tag="w0", name="w0")
                w1t = idxp.tile([128, 1024], f32, tag="w1", name="w1t")
                w2 = idxp.tile([128, 1024], i32, tag="w2", name="w2")
                w3 = idxp.tile([128, 1024], f32, tag="w3", name="w3")

                def axis(iw, ih, ga, gb, ge, qf_t):
                    # clipped affine -> exact floor (rounding-proof via is_gt)
                    nc.vector.tensor_scalar(w0[:], iw[:], grp[ga][:, kk], None,
                                            op0=OP.mult)
                    nc.vector.scalar_tensor_tensor(w0[:], ih[:],
                                                   grp[gb][:, kk], w0[:],
                                                   op0=OP.mult, op1=OP.add)
                    nc.vector.tensor_scalar(w0[:], w0[:], grp[ge][:, kk], 0.0,
                                            op0=OP.add, op1=OP.max)
                    nc.vector.tensor_scalar(w0[:], w0[:], 127.0, None,
                                            op0=OP.min)
                    nc.vector.tensor_copy(w2[:], w0[:])      # int(trunc)
                    nc.vector.tensor_copy(qf_t[:], w2[:])    # back to f32
                    w2f = w2.bitcast(f32)
                    nc.vector.tensor_tensor(w2f[:], qf_t[:], w0[:],
                                            op=OP.is_gt)
                    nc.vector.tensor_tensor(qf_t[:], qf_t[:], w2f[:],
                                            op=OP.subtract)
                    return qf_t

                x0f = axis(iw_wr_t, ih_wr_t, "ax", "bx", "ex", w1t)
                y0f = axis(iw_wr_t, ih_wr_t, "bxn", "ax", "ey", w3)
                # base = y0*128 + x0 (exact integers in f32)
                nc.vector.scalar_tensor_tensor(w0[:], y0f[:], 128.0, x0f[:],
                                               op0=OP.mult, op1=OP.add)
                nc.vector.tensor_copy(w2[:], w0[:])
                bi16 = bi16p.tile([128, 1024], i16, tag="bi16", name="bi16")
                nc.vector.tensor_copy(bi16[:], w2[:])
                return bi16

            def postprocess(q, G):
                """Reshape taps, compute bilinear weights, lerp, store: the
                8 channels of call q.

                All half-0 tap reshapes are emitted before any half-1 ones,
                so the Sync DMA queue drains the half-0 reads of G during
                gather(q, h1) and the next call's half-0 gather can start
                immediately after."""
                j = q % 8
                T4s = [tapp.tile([128, 512], bf16, tag=f"T4_{g}", name="T4")
                       for g in range(8)]
                for h in range(2):
                    for g in range(8):
                        p0 = 16 * g + 2 * j
                        for b in range(2):
                            nc.sync.dma_start(
                                T4s[g][64 * h : 64 * h + 64,
                                       ds(256 * b, 256)],
                                G[h][p0 + b : p0 + b + 1, :],
                            )
                for g in range(8):
                    c = 8 * q + g
                    cc = ds(c, 1)
                    TF = wtsp.tile([128, 512], f32, tag="TF", name="TF",
                                   bufs=2)
                    nc.vector.tensor_copy(TF[:], T4s[g][:])
                    T = [TF[:, 0:256:2], TF[:, 1:256:2],
                         TF[:, 256:512:2], TF[:, 257:512:2]]

                    def coords_q(scale_t, bias_t, u0, u1, u2):
                        nc.scalar.activation(u0[:], iw128_t[:], AF.Identity,
                                             bias=bias_t[:, cc],
                                             scale=scale_t[:, cc])
                        nc.vector.tensor_scalar(u0[:], u0[:], 0.0, 127.0,
                                                op0=OP.max, op1=OP.min)
                        nc.vector.tensor_copy(u2[:], u0[:])   # i32 trunc
                        nc.vector.tensor_copy(u1[:], u2[:])   # f32
                        u2f = u2.bitcast(f32)
                        nc.vector.tensor_tensor(u2f[:], u1[:], u0[:],
                                                op=OP.is_gt)
                        nc.vector.tensor_tensor(u1[:], u1[:], u2f[:],
                                                op=OP.subtract)  # floor
                        nc.vector.tensor_tensor(u0[:], u0[:], u1[:],
                                                op=OP.subtract)  # frac
                        return u0

                    u0 = wtsp.tile([128, 128], f32, tag="u0", name="u0")
                    u1 = wtsp.tile([128, 128], f32, tag="u1", name="u1")
                    u2 = wtsp.tile([128, 128], i32, tag="u2", name="u2")
                    fx = coords_q(ax_b, bxh_ex, u0, u1, u2)
                    u3 = wtsp.tile([128, 128], f32, tag="u3", name="u3")
                    u4 = wtsp.tile([128, 128], f32, tag="u4", name="u4")
                    u5 = wtsp.tile([128, 128], i32, tag="u5", name="u5")
                    fy = coords_q(bxn_b, axh_ey, u3, u4, u5)

                    gx0 = wtsp.tile([128, 128], f32, tag="gx0", name="gx0")
                    nc.vector.tensor_scalar(gx0[:], fx[:], -1.0, 1.0,
                                            op0=OP.mult, op1=OP.add)
                    gy0 = wtsp.tile([128, 128], f32, tag="gy0", name="gy0")
                    nc.vector.tensor_scalar(gy0[:], fy[:], -1.0, 1.0,
                                            op0=OP.mult, op1=OP.add)

                    r0 = wtsp.tile([128, 128], f32, tag="r0", name="r0")
                    r1 = wtsp.tile([128, 128], f32, tag="r1", name="r1")
                    r2 = wtsp.tile([128, 128], f32, tag="r2", name="r2")
                    # top = T0*(1-fx) + T1*fx
                    nc.vector.tensor_tensor(r0[:], T[0], gx0[:], op=OP.mult)
                    nc.vector.tensor_tensor(r1[:], T[1], fx[:], op=OP.mult)
                    nc.vector.tensor_tensor(r0[:], r0[:], r1[:], op=OP.add)
                    # bot = T2*(1-fx) + T3*fx
                    nc.vector.tensor_tensor(r2[:], T[2], gx0[:], op=OP.mult)
                    nc.vector.tensor_tensor(r1[:], T[3], fx[:], op=OP.mult)
                    nc.vector.tensor_tensor(r2[:], r2[:], r1[:], op=OP.add)
                    # out = top*(1-fy) + bot*fy
                    nc.vector.tensor_tensor(r0[:], r0[:], gy0[:], op=OP.mult)
                    nc.vector.tensor_tensor(r2[:], r2[:], fy[:], op=OP.mult)
                    nc.vector.tensor_tensor(r0[:], r0[:], r2[:], op=OP.add)
                    nc.scalar.dma_start(
                        out_d[ds(c * NPIX, NPIX)].rearrange("(a b) -> a b",
                                                            a=128),
                        r0[:],
                    )

            SKIP_POST = os.environ.get("KSKIPPOST") == "1"  # timing probe
            for _rep in range(REPEAT):
                # Two half-G buffers: call q+1's half-0 gather overwrites G0
                # as soon as call q's half-0 tap reshapes have drained, while
                # half-1 is still being consumed from G1. Each holds 8192
                # gathered bf16 pair-units per partition.
                G = [gathp.tile([128, NPIX], bf16, tag="G0", name="G0"),
                     gathp.tile([128, NPIX], bf16, tag="G1", name="G1")]
                S = None
                bi16 = None
                S_box = [load_s_tile(0)]
                for q in range(CALLS):
                    S = S_box[0]
                    if q == 0:
                        bi16 = build_idx(0)
                    for h in range(2):
                        nc.gpsimd.ap_gather(
                            G[h][:, :], S[:],
                            bi16[:, h * 512 : (h + 1) * 512],
                            channels=128, num_elems=NPIX, d=2, num_idxs=HALF)
                    # Next S tile: dispatched on the GPSIMD queue directly
                    # after this tile's last gather — queue order gives the
                    # WAR for free and the transfers spread across GPSIMD's
                    # full DMA ring set.
                    if (q + 1) % 8 == 0 and q + 1 < CALLS:
                        S_box[0] = load_s_tile((q + 1) // 8)
                    if q + 1 < CALLS:
                        bi16 = build_idx(q + 1)
                    if not SKIP_POST:
                        postprocess(q, G)

    nc.compile()
    return nc


def _prepare_in_maps(feature_map, para_code, W1, b1, Ws, bs, Wr, br, Wt, bt):
    consts = _host_constants()
    Wt_re = np.concatenate([Wt[:, 0::2], Wt[:, 1::2]], axis=1)
    bt_re = np.concatenate([bt[0::2], bt[1::2]])
    common = dict(
        w1=_mm_layout(W1, 256), ws=_mm_layout(Ws, 256), wr=_mm_layout(Wr, 256),
        wt=_mm_layout(Wt_re, 512),
        b1=_col2(b1), bs=_col2(bs), br=_col2(br),
        bt=np.ascontiguousarray(bt_re.reshape(4, 128).T),
        **consts,
    )
    common = {k: np.ascontiguousarray(v, dtype=np.float32)
              for k, v in common.items()}
    import ml_dtypes
    NU = D * NPIX + 256  # pair units shipped per core
    in_maps = []
    for i in range(NCORES):
        flat = np.concatenate([
            feature_map[i].reshape(-1),
            np.zeros(FM_PAD, dtype=np.float32),
        ]).astype(ml_dtypes.bfloat16)
        fmp = np.empty(2 * NU, dtype=ml_dtypes.bfloat16)
        fmp[0::2] = flat[:NU]
        fmp[1::2] = flat[1 : NU + 1]
        m = dict(common)
        m["fm"] = fmp
        m["pc"] = _col2(para_code[i])
        in_maps.append(m)
    return in_maps


def _run(inputs, trace=False):
    from concourse.bass_utils import run_bass_kernel_spmd

    if "nc" not in _GRAPH_CACHE:
        _GRAPH_CACHE["nc"] = _build()
    nc = _GRAPH_CACHE["nc"]
    in_maps = _prepare_in_maps(**inputs)
    res = run_bass_kernel_spmd(nc, in_maps, core_ids=list(range(NCORES)),
                               trace=trace)
    out = np.stack([
        np.asarray(res.results[i]["out"]).reshape(D, H, W)
        for i in range(NCORES)
    ])
    return out, res


def kernel(**inputs) -> np.ndarray:
    out, _ = _run(inputs, trace=False)
    return out
